# revision 1
# baseline (speedup 1.0000x reference)
"""Trainium2 Bass kernel for KeyValueAttention (4-head masked attention, gated combine).

Strategy (8 NeuronCores, query-dim sharded, 512 queries/core):
  Everything on-device runs in "transposed space" (keys/features on partitions,
  queries on the free dim), which lets both attention matmuls run without any
  on-chip transpose of the big attention matrix:
    scores^T[k,q] = K[k,:] @ Q^T        (lhsT = K^T slice, rhs = Q^T)
    E = exp(scores/8)  (ACT, fused scale, psum->sbuf bf16)
    EM = E * mask^T    (DVE, bf16 2x mode; mask DMA-cast int32->bf16)
    AV: psum[65,512] += Vaug^T_chunk.T @ EM  where Vaug = [V | ones]
        -> rows 0..63 numerator, row 64 = softmax denominator.
  Epilogue: P_h = [num_h; den_h].T @ [Wo | e_col] gives per-query denominator on
  partitions; combine heads with per-partition scalars gate_h/den_h on DVE.

Host side only reshapes/slices/transposes inputs (no reference math on host).
"""

import os
import numpy as np

NQ, NK, DC, A, H, DO = 4096, 8192, 256, 64, 4, 256
NCORES = 8
NQC = NQ // NCORES  # 512 queries per core
KC = 128            # keys per chunk
NKC = NK // KC      # 64 chunks
KBLK = 512          # keys per build block
NBLK = NK // KBLK   # 16 build blocks

_cache = {}


def _build_kernel():
    import concourse.bacc as bacc
    import concourse.mybir as mybir
    from concourse.tile import TileContext
    from concourse.masks import make_identity

    F32 = mybir.dt.float32
    BF16 = mybir.dt.bfloat16
    I32 = mybir.dt.int32
    AF = mybir.ActivationFunctionType
    ALU = mybir.AluOpType

    nc = bacc.Bacc(None, target_bir_lowering=False, debug=False)

    xqt = nc.dram_tensor("xqt", [DC, NQC], F32, kind="ExternalInput")
    maskt = nc.dram_tensor("maskt", [NK, NQC], I32, kind="ExternalInput")
    xkt = nc.dram_tensor("xkt", [DC, NK], F32, kind="ExternalInput")
    wq = nc.dram_tensor("wq", [H, DC, A], F32, kind="ExternalInput")
    wk = nc.dram_tensor("wk", [H, DC, A], F32, kind="ExternalInput")
    wv = nc.dram_tensor("wv", [H, DC, A], F32, kind="ExternalInput")
    wgt = nc.dram_tensor("wgt", [DC, H], F32, kind="ExternalInput")
    bg = nc.dram_tensor("bg", [H, 1], F32, kind="ExternalInput")
    wo = nc.dram_tensor("wo", [A, DO], F32, kind="ExternalInput")
    bo = nc.dram_tensor("bo", [1, DO], F32, kind="ExternalInput")
    out = nc.dram_tensor("out", [NQC, DO], F32, kind="ExternalOutput")

    with TileContext(nc) as tc:
        with tc.sbuf_pool(name="consts", bufs=1) as cpool:
            # Per-head-pair projection weights, layout [p, c2, (h a)]
            wqp, wkp, wvp = [], [], []
            for name, dram, lst in (("wq", wq, wqp), ("wk", wk, wkp), ("wv", wv, wvp)):
                for pr in range(2):
                    t = cpool.tile([128, 2, 2, A], F32, name=f"{name}p{pr}")
                    for hh in range(2):
                        nc.sync.dma_start(
                            t[:, :, hh, :],
                            dram[2 * pr + hh].rearrange("(c2 p) a -> p c2 a", p=128),
                        )
                    lst.append(t)
            wgt_t = cpool.tile([128, 2, H], F32)
            nc.sync.dma_start(wgt_t, wgt.rearrange("(c2 p) h -> p c2 h", p=128))
            bg_t = cpool.tile([H, 1], F32)
            nc.sync.dma_start(bg_t, bg[:])
            xqt_t = cpool.tile([128, 2, NQC], F32)
            nc.sync.dma_start(xqt_t, xqt.rearrange("(c2 p) q -> p c2 q", p=128))
            bo_t = cpool.tile([1, DO], F32)
            nc.sync.dma_start(bo_t, bo[:])
            wo_t = cpool.tile([A, DO], F32)
            nc.sync.dma_start(wo_t, wo[:])
            # wo augmented with an extra unit column that passes the denominator
            # (row 64 of the AV accumulator) through to output column 256.
            woaug = cpool.tile([A + 1, DO + 1], F32)
            nc.any.memset(woaug, 0.0)
            nc.any.tensor_copy(woaug[:A, :DO], wo_t)
            nc.any.memset(woaug[A : A + 1, DO : DO + 1], 1.0)
            ones1 = cpool.tile([1, 128], F32)
            nc.any.memset(ones1, 1.0)
            identity = cpool.tile([128, 128], F32)
            make_identity(nc, identity)

            # K^T per head pair: [128p = (2h x 64a), NK] bf16
            kt = [cpool.tile([128, NK], BF16, name=f"kt{pr}") for pr in range(2)]
            qt = [cpool.tile([128, NQC], BF16, name=f"qt{pr}") for pr in range(2)]
            # V augmented: per head [128, NKC, A+1] bf16, col A == 1.0
            vaug = [cpool.tile([128, NKC, A + 1], BF16, name=f"vaug{h}") for h in range(H)]
            for h in range(H):
                nc.any.memset(vaug[h], 1.0)
            gates = cpool.tile([H, NQC], F32)

            # ---------------- build phase ----------------
            with (
                tc.psum_pool(name="pb", bufs=1) as pb,
                tc.sbuf_pool(name="xs", bufs=2) as xs,
            ):
                # Q^T and gates
                for pr in range(2):
                    qt_ps = pb.tile([128, NQC], F32, tag="qtps", bufs=1)
                    for c2 in range(2):
                        nc.tensor.matmul(
                            qt_ps, wqp[pr][:, c2], xqt_t[:, c2],
                            start=(c2 == 0), stop=(c2 == 1),
                        )
                    nc.any.tensor_copy(qt[pr], qt_ps)
                g_ps = pb.tile([H, NQC], F32, tag="gps", bufs=1)
                for c2 in range(2):
                    nc.tensor.matmul(
                        g_ps, wgt_t[:, c2], xqt_t[:, c2],
                        start=(c2 == 0), stop=(c2 == 1),
                    )
                nc.scalar.activation(gates, g_ps, AF.Sigmoid, bias=bg_t[:], scale=1.0)

                # K^T and V over 16 key blocks
                for blk in range(NBLK):
                    xkt_t = xs.tile([128, 2, KBLK], F32, tag="xkt")
                    nc.sync.dma_start(
                        xkt_t,
                        xkt.rearrange("(c2 p) k -> p c2 k", p=128)[
                            :, :, blk * KBLK : (blk + 1) * KBLK
                        ],
                    )
                    for pr in range(2):
                        kt_ps = pb.tile([128, KBLK], F32, tag="ktps", bufs=2)
                        for c2 in range(2):
                            nc.tensor.matmul(
                                kt_ps, wkp[pr][:, c2], xkt_t[:, c2],
                                start=(c2 == 0), stop=(c2 == 1),
                            )
                        nc.any.tensor_copy(
                            kt[pr][:, blk * KBLK : (blk + 1) * KBLK], kt_ps
                        )
                    v_ps = pb.tile([128, 4, 2 * 2 * A], F32, tag="vps", bufs=2)
                    for k4 in range(4):
                        kchunk = blk * 4 + k4
                        for pr in range(2):
                            for c2 in range(2):
                                nc.tensor.matmul(
                                    v_ps[:, k4, pr * 128 : pr * 128 + 128],
                                    xkt_t[:, c2, k4 * 128 : k4 * 128 + 128],
                                    wvp[pr][:, c2],
                                    start=(c2 == 0), stop=(c2 == 1),
                                )
                    for h in range(H):
                        nc.any.tensor_copy(
                            vaug[h][:, blk * 4 : blk * 4 + 4, 0:A],
                            v_ps[:, :, h * A : h * A + A],
                        )

            # ---------------- main attention loop ----------------
            with (
                tc.psum_pool(name="pav", bufs=1) as pav,
                tc.sbuf_pool(name="ms", bufs=1) as ms,
            ):
                av = [pav.tile([A + 1, NQC], F32, name=f"av{h}", tag=f"av{h}") for h in range(H)]
                with tc.psum_pool(name="ps4", bufs=1) as ps4:
                    for kc in range(NKC):
                        mask_bf = ms.tile([128, NQC], BF16, tag="m", bufs=3)
                        nc.gpsimd.dma_start(
                            mask_bf, maskt[kc * KC : (kc + 1) * KC, :]
                        )
                        s4 = ps4.tile([128, H * NQC], F32, tag="s4", bufs=1)
                        s4v = s4.rearrange("p (h q) -> p h q", h=H)
                        for h in range(H):
                            pr, hh = h // 2, h % 2
                            nc.tensor.matmul(
                                s4v[:, h],
                                kt[pr][hh * 64 : hh * 64 + 64, kc * KC : (kc + 1) * KC],
                                qt[pr][hh * 64 : hh * 64 + 64, :],
                                start=True, stop=True,
                            )
                        e4 = ms.tile([128, H * NQC], BF16, tag="e", bufs=2)
                        nc.scalar.activation(e4, s4, AF.Exp, bias=0.0, scale=0.125)
                        em4 = ms.tile([128, H * NQC], BF16, tag="em", bufs=2)
                        e4v = e4.rearrange("p (h q) -> p h q", h=H)
                        em4v = em4.rearrange("p (h q) -> p h q", h=H)
                        nc.any.tensor_mul(
                            em4v, e4v,
                            mask_bf[:, None, :].broadcast_to([128, H, NQC]),
                        )
                        for h in range(H):
                            nc.tensor.matmul(
                                av[h],
                                vaug[h][:, kc],
                                em4v[:, h],
                                start=(kc == 0), stop=(kc == NKC - 1),
                            )

                # ---------------- epilogue ----------------
                with tc.psum_pool(name="pe", bufs=1) as pe:
                    nh = []
                    for h in range(H):
                        t = ms.tile([A + 1, NQC], F32, tag=f"nh{h}", bufs=1, name=f"nh{h}")
                        nc.any.tensor_copy(t, av[h])
                        nh.append(t)
                    gt_ps = pe.tile([128, 4 * H], F32, tag="gt", bufs=1)
                    for qtile in range(4):
                        nc.tensor.transpose(
                            gt_ps[:, qtile * H : qtile * H + H],
                            gates[:, qtile * 128 : (qtile + 1) * 128],
                            identity[:H, :H],
                        )
                    gt_sb = ms.tile([128, 4 * H], F32, tag="gtsb", bufs=1)
                    nc.any.tensor_copy(gt_sb, gt_ps)
                    boB_ps = pe.tile([128, DO], F32, tag="bob", bufs=1)
                    nc.tensor.matmul(boB_ps, ones1, bo_t, start=True, stop=True)
                    boB = ms.tile([128, DO], F32, tag="bobsb", bufs=1)
                    nc.any.tensor_copy(boB, boB_ps)
                    for qtile in range(4):
                        acc = boB
                        for h in range(H):
                            p_ps = pe.tile([128, DO + 1], F32, tag="p", bufs=2)
                            nc.tensor.matmul(
                                p_ps,
                                nh[h][:, qtile * 128 : (qtile + 1) * 128],
                                woaug,
                                start=True, stop=True,
                            )
                            rden = ms.tile([128, 1], F32, tag="rden", bufs=2)
                            nc.vector.reciprocal(rden, p_ps[:, DO : DO + 1])
                            sc = ms.tile([128, 1], F32, tag="sc", bufs=2)
                            nc.any.tensor_mul(
                                sc, rden, gt_sb[:, qtile * H + h : qtile * H + h + 1]
                            )
                            nxt = ms.tile([128, DO], F32, tag=f"acc{h % 2}", bufs=2)
                            nc.vector.scalar_tensor_tensor(
                                nxt, p_ps[:, :DO], sc, acc,
                                op0=ALU.mult, op1=ALU.add,
                            )
                            acc = nxt
                        nc.sync.dma_start(
                            out[qtile * 128 : (qtile + 1) * 128, :], acc
                        )
    nc.finalize()
    return nc


def kernel(x_Q, x_K, mask, Wq, Wk, Wv, Wg, bg, Wo, bo):
    from concourse.bass_utils import run_bass_kernel_spmd

    x_Q = np.ascontiguousarray(np.asarray(x_Q, dtype=np.float32))
    x_K = np.ascontiguousarray(np.asarray(x_K, dtype=np.float32))
    mask = np.ascontiguousarray(np.asarray(mask, dtype=np.int32))
    Wq = np.ascontiguousarray(np.asarray(Wq, dtype=np.float32))
    Wk = np.ascontiguousarray(np.asarray(Wk, dtype=np.float32))
    Wv = np.ascontiguousarray(np.asarray(Wv, dtype=np.float32))
    Wg = np.ascontiguousarray(np.asarray(Wg, dtype=np.float32))
    bg = np.asarray(bg, dtype=np.float32).reshape(H, 1)
    Wo = np.ascontiguousarray(np.asarray(Wo, dtype=np.float32))
    bo = np.asarray(bo, dtype=np.float32).reshape(1, DO)

    xkt = np.ascontiguousarray(x_K.T)
    wgt = np.ascontiguousarray(Wg.T)

    in_maps = []
    for c in range(NCORES):
        sl = slice(c * NQC, (c + 1) * NQC)
        in_maps.append({
            "xqt": np.ascontiguousarray(x_Q[sl].T),
            "maskt": np.ascontiguousarray(mask[sl].T),
            "xkt": xkt,
            "wq": Wq, "wk": Wk, "wv": Wv,
            "wgt": wgt, "bg": bg, "wo": Wo, "bo": bo,
        })

    if "nc" not in _cache:
        _cache["nc"] = _build_kernel()
    res = run_bass_kernel_spmd(
        _cache["nc"], in_maps, list(range(NCORES)),
        trace=bool(int(os.environ.get("BASS_KERNEL_TRACE", "0"))),
    )
    if res.exec_time_ns is not None:
        print(f"HW exec time: {res.exec_time_ns} ns")
    return np.concatenate([r["out"] for r in res.results], axis=0)



# revision 16
# speedup vs baseline: 1.5459x; 1.5459x over previous
"""Trainium2 Bass kernel for KeyValueAttention (4-head masked attention, gated combine).

v3 strategy (8 NeuronCores, query-dim sharded, 512 queries/core):
  Transposed space throughout (keys/features on partitions, queries on free dim).
  - All projections (Q/K/V) are fp8e4 DoubleRow matmuls (contraction 256 as
    2x128 k-tiles) -> 0.5 cycles/row on the PE.
  - TWO PASSES over the keys, one per head pair. Per pass the scores psum
    rotates through 3 buffers (6 banks) and the 2 AV accumulators use 2 banks,
    fitting the 8-bank PSUM while keeping the exp pipeline deep.
  - Scores: fp8 DR matmul, contraction A=64 as 2x32 k-tiles:
    lhsT = K^T chunk [32, 2, 128], rhs = Q^T [32, 2, 512] -> psum [128k, 512q].
  - Masked exp alternates engines by chunk parity:
      * even chunks (ACT): mask pre-added as -160 bias via an identity DR
        matmul opening the psum accumulation group, then ACT Exp (scale=1/8).
      * odd chunks (DVE): custom DVE op computes cubic-poly exp(s/8) * mask
        stream in one pass (Src0 = psum scores, Src1 = fp8 mask from SBUF).
    Both write em directly as fp8e4.
  - The fp8 mask image for all chunks is DMA'd into SBUF once (pass 1) and
    reused from SBUF in pass 2.
  - AV: fp8 DR over chunk pairs: lhsT = Vaug [128, 2, 65], rhs = em
    [128, 2, 512] -> psum [65, 512] per head; row 64 = softmax denominator.
  - The pass-2 K/V build matmuls are interleaved into the pass-1 chunk loop.

Host side only reshapes/slices/transposes/casts inputs (no reference math).
"""

import os
import numpy as np

NQ, NK, DC, A, H, DO = 4096, 8192, 256, 64, 4, 256
NCORES = 8
NQC = NQ // NCORES   # 512 queries per core
KC = 128             # keys per chunk
NKC = NK // KC       # 64 chunks
NPAIR = NKC // 2     # 32 chunk pairs

# chunk -> engine map: True = ACT chunk (mask via additive bias),
# False = DVE chunk (mask via Src1 stream in the custom op).
CHUNK_IS_ACT = [bool(c % 2 == 0) for c in range(NKC)]

MASK_BIAS = -160.0  # additive pre-scale bias for masked keys (ACT chunks)

_cache = {}


# ---------------------------------------------------------------------------
# exp polynomial fit (shared host/device constants)
# ---------------------------------------------------------------------------
def _fit_exp_poly(scale=0.125, lo=-0.85, hi=0.85):
    """p(x) = 1 + b1 x + b2 x^2 + b3 x^3 ~ exp(x*scale) for x*scale in [lo,hi],
    relative-error weighted, p(0)=1 pinned."""
    t = np.linspace(lo, hi, 40001)
    w = 1.0 / np.exp(t)
    Amat = np.stack([t, t * t, t ** 3], axis=1) * w[:, None]
    a = np.linalg.lstsq(Amat, (np.exp(t) - 1.0) * w, rcond=None)[0]
    return [float(a[0] * scale), float(a[1] * scale ** 2), float(a[2] * scale ** 3)]


POLY_B = _fit_exp_poly()


def _register_dve_exp_op():
    """Define + register the custom DVE op (idempotent)."""
    from concourse.dve_spec import Spec, Src0, Src1, C0, C1, C2, One, lower
    from concourse.dve_ops import (
        DveOp, OPS, CUSTOM_DVE_SPECS, _SUB_OPCODE_FOR_NAME, _CUSTOM_DVE_ROW_BASE,
    )
    from concourse.dve_table_gen import dve_ver_for
    from concourse.dve_uop import DveOpSpec

    name = "EXP_POLY_MASK_ANT"
    if name in _SUB_OPCODE_FOR_NAME:
        return next(op for op in OPS if op.name == name)

    body = (((Src0 * C2 + C1) * Src0 + C0) * Src0 + One) * Src1
    spec = Spec(
        body=body,
        reference=lambda in0, in1, s0, s1, imm2: (
            (((in0 * imm2 + s1) * in0 + s0) * in0 + 1.0) * in1
        ),
    )
    op = DveOp(name, spec, subdim=False, uops_sha={})
    ver = dve_ver_for("TRN2")
    op.uops_sha[ver] = DveOpSpec(
        name=name, opcode=31, uops=lower(spec, ver=ver), rd1_en=True
    ).sha(ver)
    OPS.append(op)
    CUSTOM_DVE_SPECS[name] = spec
    _SUB_OPCODE_FOR_NAME[name] = _CUSTOM_DVE_ROW_BASE + len(OPS) - 1
    return op


# ---------------------------------------------------------------------------
# kernel build
# ---------------------------------------------------------------------------
def _build_kernel():
    import concourse.bacc as bacc
    import concourse.mybir as mybir
    from concourse.tile import TileContext
    from concourse.masks import make_identity

    EXP_OP = _register_dve_exp_op()

    F32 = mybir.dt.float32
    BF16 = mybir.dt.bfloat16
    FP8 = mybir.dt.float8e4
    AF = mybir.ActivationFunctionType
    ALU = mybir.AluOpType
    DR = mybir.MatmulPerfMode.DoubleRow

    nc = bacc.Bacc(None, target_bir_lowering=False, debug=False)

    def eng_copy(eng, dst, src):
        # NOTE: gpsimd cannot access PSUM on HW; keep psum reads on scalar/vector.
        if eng is nc.scalar:
            nc.scalar.copy(dst, src)
        else:
            eng.tensor_copy(dst, src)

    # ---- DRAM inputs (per core) ----
    xqtb = nc.dram_tensor("xqtb", [128, 2, NQC], BF16, kind="ExternalInput")
    xkt8 = nc.dram_tensor("xkt8", [128, 2, NK], FP8, kind="ExternalInput")
    wqb = nc.dram_tensor("wqb", [128, 2, H, A], BF16, kind="ExternalInput")
    wkTb = nc.dram_tensor("wkTb", [64, 2, H, 128], BF16, kind="ExternalInput")
    wv8 = nc.dram_tensor("wv8", [128, 2, H * A], FP8, kind="ExternalInput")
    wgtb = nc.dram_tensor("wgtb", [128, 2, H], BF16, kind="ExternalInput")
    bg = nc.dram_tensor("bg", [H, 1], F32, kind="ExternalInput")
    wo = nc.dram_tensor("wo", [A, DO], F32, kind="ExternalInput")
    bo = nc.dram_tensor("bo", [1, DO], F32, kind="ExternalInput")
    maskx = nc.dram_tensor("maskx", [NKC, 128, 2 * NQC], FP8, kind="ExternalInput")
    out = nc.dram_tensor("out", [NQC, DO], F32, kind="ExternalOutput")

    with TileContext(nc) as tc:
        with tc.sbuf_pool(name="consts", bufs=1) as cpool:
            # ---- constants ----
            wq_t = cpool.tile([128, 2, H, A], BF16)
            nc.sync.dma_start(wq_t, wqb[:])
            wkT_t = cpool.tile([64, 2, H, 128], BF16)
            nc.sync.dma_start(wkT_t, wkTb[:])
            wv_t = cpool.tile([128, 2, H * A], FP8)
            nc.sync.dma_start(wv_t, wv8[:])
            wgt_t = cpool.tile([128, 2, H], BF16)
            nc.sync.dma_start(wgt_t, wgtb[:])
            bg_t = cpool.tile([H, 1], F32)
            nc.sync.dma_start(bg_t, bg[:])
            xqtb_t = cpool.tile([128, 2, NQC], BF16)
            nc.sync.dma_start(xqtb_t, xqtb[:])
            xkt_t = cpool.tile([128, 2, NK], FP8)
            nc.sync.dma_start(xkt_t, xkt8[:])
            bo_t = cpool.tile([1, DO], F32)
            nc.sync.dma_start(bo_t, bo[:])
            wo_t = cpool.tile([A, DO], F32)
            nc.sync.dma_start(wo_t, wo[:])
            woaug = cpool.tile([A + 1, DO + 1], BF16)
            nc.vector.memset(woaug, 0.0)
            nc.any.tensor_copy(woaug[:A, :DO], wo_t)
            nc.vector.memset(woaug[A : A + 1, DO : DO + 1], 1.0)
            ones1 = cpool.tile([1, 128], F32)
            nc.vector.memset(ones1, 1.0)
            identity = cpool.tile([128, 128], F32)
            make_identity(nc, identity)

            # E2: DR identity for the mask-add matmul.
            # E2[p, i*128 + c] = 1 iff c == i*64 + p  (p - f + 192*i == 0)
            e2bf = cpool.tile([64, 256], BF16)
            nc.gpsimd.memset(e2bf, 0.0)
            for i in range(2):
                nc.gpsimd.affine_select(
                    out=e2bf, in_=e2bf, compare_op=ALU.not_equal,
                    fill=1.0, base=192 * i, pattern=[[-1, 256]], channel_multiplier=1,
                )
            e2 = cpool.tile([64, 2, 128], FP8)
            nc.vector.tensor_copy(e2.rearrange("p i c -> p (i c)"), e2bf)

            # ---- persistent operand tiles ----
            # QW[h] = Wk_h @ Q_h^T in fp8 DR layout [128, 2, NQC] (c = i*128+p)
            qw8 = [cpool.tile([128, 2, NQC], FP8, name=f"qw{h}") for h in range(H)]
            qt_bf = cpool.tile([64, H, NQC], BF16)
            # last dim padded to 80 so the AV DoubleRow k-tile step is %16==0
            vaug = cpool.tile([128, H, NKC, 80], FP8)
            # only the augmented ones-column needs initialization
            nc.gpsimd.memset(vaug[:, :, :, A : A + 1], 1.0)
            gates = cpool.tile([H, NQC], F32)
            # whole mask image, SBUF resident (written in pass 1, reused pass 2)
            mask_sb = cpool.tile([128, NKC, 2 * NQC], FP8)
            nh = [cpool.tile([A + 1, NQC], BF16, name=f"nh{h}") for h in range(H)]

            KBLK = 512

            with (
                tc.psum_pool(name="pmain", bufs=1) as pm,
                tc.sbuf_pool(name="ms", bufs=1) as ms,
            ):
                # ---- build helpers (all ride the "sset" psum rotation) ----
                def sset_tile():
                    s4 = pm.tile([128, 2, NQC], F32, tag="sset", bufs=3,
                                 name="s4")
                    return s4

                def build_qt(hpair):
                    # Q_h^T = Wq_h^T @ x_Q^T  (bf16), heads 2*hpair, 2*hpair+1
                    qps = sset_tile()
                    for hh in range(2):
                        h = 2 * hpair + hh
                        for i in range(2):
                            nc.tensor.matmul(
                                qps[0:64, hh, :], wq_t[:, i, h, :],
                                xqtb_t[:, i, :],
                                start=(i == 0), stop=(i == 1),
                            )
                        eng_copy((nc.scalar, nc.vector)[hh], qt_bf[:, h, :],
                                 qps[0:64, hh, :])

                def build_qw(h):
                    # QW_h = Wk_h @ Q_h^T -> fp8 [128, 2, NQC] (c = i*128+p)
                    qps = sset_tile()
                    for half in range(2):
                        nc.tensor.matmul(
                            qps[:, half, :], wkT_t[:, half, h, :],
                            qt_bf[:, h, :],
                            start=True, stop=True,
                        )
                        eng_copy((nc.scalar, nc.vector)[half],
                                 qw8[h][:, half, :], qps[:, half, :])

                def build_v(P, cp):  # one key chunk PAIR, heads of pair P
                    vps = sset_tile()
                    for s in range(2):
                        c = 2 * cp + s
                        nc.tensor.matmul(
                            vps[:, s, 0 : 2 * A],
                            xkt_t[:, :, c * KC : (c + 1) * KC],
                            wv_t[:, :, 2 * P * A : (2 * P + 2) * A],
                            start=True, stop=True, perf_mode=DR,
                        )
                        eng_copy(
                            (nc.scalar, nc.vector)[s],
                            vaug[:, 2 * P : 2 * P + 2, c, 0:A],
                            vps[:, s, 0 : 2 * A].rearrange("p (h a) -> p h a", h=2),
                        )

                # ---- upfront mask DMAs (8 batched, sync engine) ----
                for g in range(8):
                    nc.sync.dma_start(
                        mask_sb[:, 8 * g : 8 * (g + 1), :],
                        maskx[8 * g : 8 * (g + 1)].rearrange("c p q -> p c q"),
                    )

                # ---- build: Q, QW (all heads), gates, V for pass 0 ----
                for hpair in range(2):
                    build_qt(hpair)
                for h in range(H):
                    build_qw(h)
                g_ps = sset_tile()
                for i in range(2):
                    nc.tensor.matmul(
                        g_ps[0:4, 0, :], wgt_t[:, i, :], xqtb_t[:, i, :],
                        start=(i == 0), stop=(i == 1),
                    )
                nc.scalar.activation(gates, g_ps[0:4, 0, :], AF.Sigmoid,
                                     bias=bg_t[:], scale=1.0)

                for cp in range(NPAIR):
                    build_v(0, cp)

                # ---- two passes over keys, one head pair each ----
                for PASS in range(2):
                    h0 = 2 * PASS
                    avP = [
                        pm.tile([A + 1, NQC], F32, tag=f"av{hh}", bufs=1,
                                name=f"av{hh}")
                        for hh in range(2)
                    ]
                    em_cur = None
                    for c in range(NKC):
                        is_act = CHUNK_IS_ACT[c]
                        pair, slot = c // 2, c % 2
                        if slot == 0:
                            em_cur = ms.tile([128, 2, 2, NQC], FP8, tag="em", bufs=2)

                        if PASS == 0 and c < 32:
                            # interleave pass-1 V build into the pass-0 loop
                            build_v(1, c)

                        s4 = sset_tile()
                        for hh in range(2):
                            h = h0 + hh
                            if is_act:
                                nc.tensor.matmul(
                                    s4[:, hh, :], e2,
                                    mask_sb[0:64, c, :].rearrange(
                                        "p (i q) -> p i q", i=2
                                    ),
                                    start=True, stop=False, perf_mode=DR,
                                )
                            nc.tensor.matmul(
                                s4[:, hh, :],
                                xkt_t[:, :, c * KC : (c + 1) * KC],
                                qw8[h],
                                start=not is_act, stop=True,
                                perf_mode=DR,
                            )
                        dst = em_cur[:, slot]
                        if is_act:
                            nc.scalar.activation(
                                dst, s4, AF.Exp, bias=0.0, scale=0.125
                            )
                        else:
                            nc.vector._custom_dve(
                                EXP_OP, out=dst, in0=s4, in1=mask_sb[:, c, :],
                                s0=POLY_B[0], s1=POLY_B[1], imm2=POLY_B[2],
                            )

                        if slot == 1:
                            for hh in range(2):
                                nc.tensor.matmul(
                                    avP[hh],
                                    vaug[:, h0 + hh, 2 * pair : 2 * pair + 2, 0 : A + 1],
                                    em_cur[:, :, hh, :],
                                    start=(pair == 0), stop=(pair == NPAIR - 1),
                                    perf_mode=DR,
                                )

                    for hh in range(2):
                        eng_copy((nc.scalar, nc.vector)[hh], nh[h0 + hh], avP[hh])

            # ---------------- epilogue ----------------
            with (
                tc.psum_pool(name="pe", bufs=1) as pm,
                tc.sbuf_pool(name="es", bufs=1) as ms,
            ):
                gt_ps = pm.tile([128, 4 * H], F32, tag="gt", bufs=1)
                for qtile in range(4):
                    nc.tensor.transpose(
                        gt_ps[:, qtile * H : qtile * H + H],
                        gates[:, qtile * 128 : (qtile + 1) * 128],
                        identity[:H, :H],
                    )
                gt_sb = ms.tile([128, 4 * H], F32, tag="gtsb", bufs=1)
                nc.any.tensor_copy(gt_sb, gt_ps)
                boB_ps = pm.tile([128, DO], F32, tag="bob", bufs=1)
                nc.tensor.matmul(boB_ps, ones1, bo_t, start=True, stop=True)
                boB = ms.tile([128, DO], F32, tag="bobsb", bufs=1)
                nc.any.tensor_copy(boB, boB_ps)
                for qtile in range(4):
                    acc = boB
                    for h in range(H):
                        p_ps = pm.tile([128, DO + 1], F32, tag="p", bufs=2)
                        nc.tensor.matmul(
                            p_ps,
                            nh[h][:, qtile * 128 : (qtile + 1) * 128],
                            woaug,
                            start=True, stop=True,
                        )
                        rden = ms.tile([128, 1], F32, tag="rden", bufs=2)
                        nc.vector.reciprocal(rden, p_ps[:, DO : DO + 1])
                        sc = ms.tile([128, 1], F32, tag="sc", bufs=2)
                        nc.any.tensor_mul(
                            sc, rden, gt_sb[:, qtile * H + h : qtile * H + h + 1]
                        )
                        nxt = ms.tile([128, DO], F32, tag=f"acc{h % 2}", bufs=2)
                        nc.vector.scalar_tensor_tensor(
                            nxt, p_ps[:, :DO], sc, acc,
                            op0=ALU.mult, op1=ALU.add,
                        )
                        acc = nxt
                    nc.sync.dma_start(
                        out[qtile * 128 : (qtile + 1) * 128, :], acc
                    )
    nc.finalize()
    return nc


# ---------------------------------------------------------------------------
# host-side input prep
# ---------------------------------------------------------------------------
def _to_f8(x):
    import ml_dtypes
    return np.ascontiguousarray(np.asarray(x, dtype=np.float32).astype(
        ml_dtypes.float8_e4m3fn))


def _to_bf16(x):
    import ml_dtypes
    return np.ascontiguousarray(np.asarray(x, dtype=np.float32).astype(
        ml_dtypes.bfloat16))


def _dr_c_layout(xT):
    """[C=256, N] -> [128, 2, N] with c = i*128 + p."""
    return np.ascontiguousarray(xT.reshape(2, 128, -1).transpose(1, 0, 2))


def _prep_shared(x_K, Wq, Wk, Wv, Wg, bg, Wo, bo):
    xkt = x_K.T  # [256, NK]
    xkt8 = _to_f8(_dr_c_layout(xkt))

    # wqb[p, i, h, a] = Wq[h, i*128+p, a]
    wqb = _to_bf16(Wq.transpose(1, 0, 2).reshape(2, 128, H, A).transpose(1, 0, 2, 3))
    # wkTb[a, half, h, m] = Wk[h, 128*half + m, a]
    wkTb = _to_bf16(
        Wk.reshape(H, 2, 128, A).transpose(3, 1, 0, 2)
    )
    arr = np.empty((128, 2, H * A), np.float32)
    for h in range(H):
        arr[:, :, h * A:(h + 1) * A] = Wv[h].reshape(2, 128, A).transpose(1, 0, 2)
    wv8 = _to_f8(arr)
    wgtb = _to_bf16(Wg.T.reshape(2, 128, H).transpose(1, 0, 2))
    return {
        "xkt8": xkt8, "wqb": wqb, "wkTb": wkTb, "wv8": wv8, "wgtb": wgtb,
        "bg": np.asarray(bg, np.float32).reshape(H, 1),
        "wo": np.ascontiguousarray(np.asarray(Wo, np.float32)),
        "bo": np.asarray(bo, np.float32).reshape(1, DO),
    }


def _prep_mask_core(mask_sl):
    """mask_sl: [NQC, NK] int32 -> maskx [NKC, 128, 2*NQC] fp8 per-chunk layout."""
    import ml_dtypes
    mt = mask_sl.T.astype(np.float32)  # [NK, NQC]
    maskx = np.zeros((NKC, 128, 2 * NQC), np.float32)
    for c in range(NKC):
        blk = mt[c * KC:(c + 1) * KC]  # [128, NQC]
        if CHUNK_IS_ACT[c]:
            # rows p<64: col i*512+q holds bias of key k = i*64+p
            bias = (blk - 1.0) * (-MASK_BIAS)
            maskx[c, 0:64, 0:NQC] = bias[0:64]
            maskx[c, 0:64, NQC:] = bias[64:128]
        else:
            maskx[c, :, 0:NQC] = blk
            maskx[c, :, NQC:] = blk
    return np.ascontiguousarray(maskx.astype(ml_dtypes.float8_e4m3fn))


def kernel(x_Q, x_K, mask, Wq, Wk, Wv, Wg, bg, Wo, bo):
    from concourse.bass_utils import run_bass_kernel_spmd

    x_Q = np.asarray(x_Q, dtype=np.float32)
    x_K = np.asarray(x_K, dtype=np.float32)
    mask = np.asarray(mask, dtype=np.int32)

    shared = _prep_shared(
        x_K, np.asarray(Wq, np.float32), np.asarray(Wk, np.float32),
        np.asarray(Wv, np.float32), np.asarray(Wg, np.float32),
        bg, Wo, bo,
    )

    in_maps = []
    for cidx in range(NCORES):
        sl = slice(cidx * NQC, (cidx + 1) * NQC)
        xqt = x_Q[sl].T  # [256, NQC]
        m = {
            "xqtb": _to_bf16(_dr_c_layout(xqt)),
            "maskx": _prep_mask_core(mask[sl]),
        }
        m.update(shared)
        in_maps.append(m)

    if "nc" not in _cache:
        _cache["nc"] = _build_kernel()
    res = run_bass_kernel_spmd(
        _cache["nc"], in_maps, list(range(NCORES)),
        trace=bool(int(os.environ.get("BASS_KERNEL_TRACE", "0"))),
    )
    if res.exec_time_ns is not None:
        print(f"HW exec time: {res.exec_time_ns} ns")
    return np.concatenate([r["out"] for r in res.results], axis=0)


# revision 17
# speedup vs baseline: 1.7578x; 1.1371x over previous
"""Trainium2 Bass kernel for KeyValueAttention (4-head masked attention, gated combine).

v3 strategy (8 NeuronCores, query-dim sharded, 512 queries/core):
  Transposed space throughout (keys/features on partitions, queries on free dim).
  - All projections (Q/K/V) are fp8e4 DoubleRow matmuls (contraction 256 as
    2x128 k-tiles) -> 0.5 cycles/row on the PE.
  - TWO PASSES over the keys, one per head pair. Per pass the scores psum
    rotates through 3 buffers (6 banks) and the 2 AV accumulators use 2 banks,
    fitting the 8-bank PSUM while keeping the exp pipeline deep.
  - Scores: fp8 DR matmul, contraction A=64 as 2x32 k-tiles:
    lhsT = K^T chunk [32, 2, 128], rhs = Q^T [32, 2, 512] -> psum [128k, 512q].
  - Masked exp alternates engines by chunk parity:
      * even chunks (ACT): mask pre-added as -160 bias via an identity DR
        matmul opening the psum accumulation group, then ACT Exp (scale=1/8).
      * odd chunks (DVE): custom DVE op computes cubic-poly exp(s/8) * mask
        stream in one pass (Src0 = psum scores, Src1 = fp8 mask from SBUF).
    Both write em directly as fp8e4.
  - The fp8 mask image for all chunks is DMA'd into SBUF once (pass 1) and
    reused from SBUF in pass 2.
  - AV: fp8 DR over chunk pairs: lhsT = Vaug [128, 2, 65], rhs = em
    [128, 2, 512] -> psum [65, 512] per head; row 64 = softmax denominator.
  - The pass-2 K/V build matmuls are interleaved into the pass-1 chunk loop.

Host side only reshapes/slices/transposes/casts inputs (no reference math).
"""

import os
import numpy as np

NQ, NK, DC, A, H, DO = 4096, 8192, 256, 64, 4, 256
NCORES = 8
NQC = NQ // NCORES   # 512 queries per core
KC = 128             # keys per chunk
NKC = NK // KC       # 64 chunks
NPAIR = NKC // 2     # 32 chunk pairs

# chunk -> engine map: True = ACT chunk (mask via additive bias),
# False = DVE chunk (mask via Src1 stream in the custom op).
CHUNK_IS_ACT = [bool(c % 2 == 0) for c in range(NKC)]

MASK_BIAS = -160.0  # additive pre-scale bias for masked keys (ACT chunks)

_cache = {}


# ---------------------------------------------------------------------------
# exp polynomial fit (shared host/device constants)
# ---------------------------------------------------------------------------
def _fit_exp_poly(scale=0.125, lo=-0.85, hi=0.85):
    """p(x) = 1 + b1 x + b2 x^2 + b3 x^3 ~ exp(x*scale) for x*scale in [lo,hi],
    relative-error weighted, p(0)=1 pinned."""
    t = np.linspace(lo, hi, 40001)
    w = 1.0 / np.exp(t)
    Amat = np.stack([t, t * t, t ** 3], axis=1) * w[:, None]
    a = np.linalg.lstsq(Amat, (np.exp(t) - 1.0) * w, rcond=None)[0]
    return [float(a[0] * scale), float(a[1] * scale ** 2), float(a[2] * scale ** 3)]


POLY_B = _fit_exp_poly()


def _register_dve_exp_op():
    """Define + register the custom DVE op (idempotent)."""
    from concourse.dve_spec import Spec, Src0, Src1, C0, C1, C2, One, lower
    from concourse.dve_ops import (
        DveOp, OPS, CUSTOM_DVE_SPECS, _SUB_OPCODE_FOR_NAME, _CUSTOM_DVE_ROW_BASE,
    )
    from concourse.dve_table_gen import dve_ver_for
    from concourse.dve_uop import DveOpSpec

    name = "EXP_POLY_MASK_ANT"
    if name in _SUB_OPCODE_FOR_NAME:
        return next(op for op in OPS if op.name == name)

    body = (((Src0 * C2 + C1) * Src0 + C0) * Src0 + One) * Src1
    spec = Spec(
        body=body,
        reference=lambda in0, in1, s0, s1, imm2: (
            (((in0 * imm2 + s1) * in0 + s0) * in0 + 1.0) * in1
        ),
    )
    op = DveOp(name, spec, subdim=False, uops_sha={})
    ver = dve_ver_for("TRN2")
    op.uops_sha[ver] = DveOpSpec(
        name=name, opcode=31, uops=lower(spec, ver=ver), rd1_en=True
    ).sha(ver)
    OPS.append(op)
    CUSTOM_DVE_SPECS[name] = spec
    _SUB_OPCODE_FOR_NAME[name] = _CUSTOM_DVE_ROW_BASE + len(OPS) - 1
    return op


# ---------------------------------------------------------------------------
# kernel build
# ---------------------------------------------------------------------------
def _build_kernel():
    import concourse.bacc as bacc
    import concourse.mybir as mybir
    from concourse.tile import TileContext
    from concourse.masks import make_identity

    EXP_OP = _register_dve_exp_op()

    F32 = mybir.dt.float32
    BF16 = mybir.dt.bfloat16
    FP8 = mybir.dt.float8e4
    AF = mybir.ActivationFunctionType
    ALU = mybir.AluOpType
    DR = mybir.MatmulPerfMode.DoubleRow

    nc = bacc.Bacc(None, target_bir_lowering=False, debug=False)

    def eng_copy(eng, dst, src):
        # NOTE: gpsimd cannot access PSUM on HW; keep psum reads on scalar/vector.
        if eng is nc.scalar:
            nc.scalar.copy(dst, src)
        else:
            eng.tensor_copy(dst, src)

    # ---- DRAM inputs (per core) ----
    xqtb = nc.dram_tensor("xqtb", [128, 2, NQC], BF16, kind="ExternalInput")
    xkt8 = nc.dram_tensor("xkt8", [128, 2, NK], FP8, kind="ExternalInput")
    wqb = nc.dram_tensor("wqb", [128, 2, H, A], BF16, kind="ExternalInput")
    wkTb = nc.dram_tensor("wkTb", [64, 2, H, 128], BF16, kind="ExternalInput")
    wv8 = nc.dram_tensor("wv8", [128, 2, H * A], FP8, kind="ExternalInput")
    wgtb = nc.dram_tensor("wgtb", [128, 2, H], BF16, kind="ExternalInput")
    bg = nc.dram_tensor("bg", [H, 1], F32, kind="ExternalInput")
    wo = nc.dram_tensor("wo", [A, DO], F32, kind="ExternalInput")
    bo = nc.dram_tensor("bo", [1, DO], F32, kind="ExternalInput")
    maskx = nc.dram_tensor("maskx", [NKC, 128, 2 * NQC], FP8, kind="ExternalInput")
    out = nc.dram_tensor("out", [NQC, DO], F32, kind="ExternalOutput")

    with TileContext(nc) as tc:
        with tc.sbuf_pool(name="consts", bufs=1) as cpool:
            # ---- constants ----
            wq_t = cpool.tile([128, 2, H, A], BF16)
            nc.sync.dma_start(wq_t, wqb[:])
            wkT_t = cpool.tile([64, 2, H, 128], BF16)
            nc.sync.dma_start(wkT_t, wkTb[:])
            wv_t = cpool.tile([128, 2, H * A], FP8)
            nc.sync.dma_start(wv_t, wv8[:])
            wgt_t = cpool.tile([128, 2, H], BF16)
            nc.sync.dma_start(wgt_t, wgtb[:])
            bg_t = cpool.tile([H, 1], F32)
            nc.sync.dma_start(bg_t, bg[:])
            xqtb_t = cpool.tile([128, 2, NQC], BF16)
            nc.sync.dma_start(xqtb_t, xqtb[:])
            xkt_t = cpool.tile([128, 2, NK], FP8)
            nc.sync.dma_start(xkt_t, xkt8[:])
            bo_t = cpool.tile([1, DO], F32)
            nc.sync.dma_start(bo_t, bo[:])
            wo_t = cpool.tile([A, DO], F32)
            nc.sync.dma_start(wo_t, wo[:])
            woaug = cpool.tile([A + 1, DO + 1], BF16)
            nc.vector.memset(woaug, 0.0)
            nc.any.tensor_copy(woaug[:A, :DO], wo_t)
            nc.vector.memset(woaug[A : A + 1, DO : DO + 1], 1.0)
            ones1 = cpool.tile([1, 128], F32)
            nc.vector.memset(ones1, 1.0)
            identity = cpool.tile([128, 128], F32)
            make_identity(nc, identity)

            # E2: DR identity for the mask-add matmul.
            # E2[p, i*128 + c] = 1 iff c == i*64 + p  (p - f + 192*i == 0)
            e2bf = cpool.tile([64, 256], BF16)
            nc.gpsimd.memset(e2bf, 0.0)
            for i in range(2):
                nc.gpsimd.affine_select(
                    out=e2bf, in_=e2bf, compare_op=ALU.not_equal,
                    fill=1.0, base=192 * i, pattern=[[-1, 256]], channel_multiplier=1,
                )
            e2 = cpool.tile([64, 2, 128], FP8)
            nc.vector.tensor_copy(e2.rearrange("p i c -> p (i c)"), e2bf)

            # ---- persistent operand tiles ----
            # QW[h] = Wk_h @ Q_h^T in fp8 DR layout [128, 2, NQC] (c = i*128+p)
            qw8 = [cpool.tile([128, 2, NQC], FP8, name=f"qw{h}") for h in range(H)]
            qt_bf = cpool.tile([64, H, NQC], BF16)
            # last dim padded to 80 so the AV DoubleRow k-tile step is %16==0
            vaug = cpool.tile([128, H, NKC, 80], FP8)
            # only the augmented ones-column needs initialization
            nc.gpsimd.memset(vaug[:, :, :, A : A + 1], 1.0)
            gates = cpool.tile([H, NQC], F32)
            # whole mask image, SBUF resident (written in pass 1, reused pass 2)
            mask_sb = cpool.tile([128, NKC, 2 * NQC], FP8)
            nh = [cpool.tile([A + 1, NQC], BF16, name=f"nh{h}") for h in range(H)]

            KBLK = 512

            with (
                tc.psum_pool(name="pmain", bufs=1) as pm,
                tc.sbuf_pool(name="ms", bufs=1) as ms,
            ):
                # ---- build helpers (all ride the "sset" psum rotation) ----
                def sset_tile():
                    s4 = pm.tile([128, 2, NQC], F32, tag="sset", bufs=3,
                                 name="s4")
                    return s4

                def build_qt(hpair):
                    # Q_h^T = Wq_h^T @ x_Q^T  (bf16), heads 2*hpair, 2*hpair+1
                    qps = sset_tile()
                    for hh in range(2):
                        h = 2 * hpair + hh
                        for i in range(2):
                            nc.tensor.matmul(
                                qps[0:64, hh, :], wq_t[:, i, h, :],
                                xqtb_t[:, i, :],
                                start=(i == 0), stop=(i == 1),
                            )
                        eng_copy((nc.scalar, nc.vector)[hh], qt_bf[:, h, :],
                                 qps[0:64, hh, :])

                def build_qw(h):
                    # QW_h = Wk_h @ Q_h^T -> fp8 [128, 2, NQC] (c = i*128+p)
                    qps = sset_tile()
                    for half in range(2):
                        nc.tensor.matmul(
                            qps[:, half, :], wkT_t[:, half, h, :],
                            qt_bf[:, h, :],
                            start=True, stop=True,
                        )
                        eng_copy((nc.scalar, nc.vector)[half],
                                 qw8[h][:, half, :], qps[:, half, :])

                def build_v(P, cp):  # one key chunk PAIR, heads of pair P
                    vps = sset_tile()
                    for s in range(2):
                        c = 2 * cp + s
                        nc.tensor.matmul(
                            vps[:, s, 0 : 2 * A],
                            xkt_t[:, :, c * KC : (c + 1) * KC],
                            wv_t[:, :, 2 * P * A : (2 * P + 2) * A],
                            start=True, stop=True, perf_mode=DR,
                        )
                        eng_copy(
                            (nc.scalar, nc.vector)[s],
                            vaug[:, 2 * P : 2 * P + 2, c, 0:A],
                            vps[:, s, 0 : 2 * A].rearrange("p (h a) -> p h a", h=2),
                        )

                # ---- upfront mask DMAs (8 batched, sync engine) ----
                for g in range(8):
                    nc.sync.dma_start(
                        mask_sb[:, 8 * g : 8 * (g + 1), :],
                        maskx[8 * g : 8 * (g + 1)].rearrange("c p q -> p c q"),
                    )

                # ---- build: Q, QW (all heads), gates, V for pass 0 ----
                for hpair in range(2):
                    build_qt(hpair)
                for h in range(H):
                    build_qw(h)
                g_ps = sset_tile()
                for i in range(2):
                    nc.tensor.matmul(
                        g_ps[0:4, 0, :], wgt_t[:, i, :], xqtb_t[:, i, :],
                        start=(i == 0), stop=(i == 1),
                    )
                nc.scalar.activation(gates, g_ps[0:4, 0, :], AF.Sigmoid,
                                     bias=bg_t[:], scale=1.0)

                for cp in range(NPAIR):
                    build_v(0, cp)

                # ---- two passes over keys, one head pair each ----
                for PASS in range(2):
                    h0 = 2 * PASS
                    avP = [
                        pm.tile([A + 1, NQC], F32, tag=f"av{hh}", bufs=1,
                                name=f"av{hh}")
                        for hh in range(2)
                    ]
                    for pair in range(NPAIR):
                        c0, c1 = 2 * pair, 2 * pair + 1  # c0 = ACT, c1 = DVE
                        em_cur = ms.tile([128, 2, 2, NQC], FP8, tag="em", bufs=2)

                        if PASS == 0 and pair < NPAIR // 2:
                            # interleave pass-1 V build into the pass-0 loop
                            build_v(1, 2 * pair)
                            build_v(1, 2 * pair + 1)

                        s4a = sset_tile()
                        s4d = sset_tile()
                        # mask-add for the ACT chunk (both heads, 2 banks)
                        for hh in range(2):
                            nc.tensor.matmul(
                                s4a[:, hh, :], e2,
                                mask_sb[0:64, c0, :].rearrange(
                                    "p (i q) -> p i q", i=2
                                ),
                                start=True, stop=False, perf_mode=DR,
                            )
                        # independent DVE-chunk scores separate the dependent
                        # mask-add -> scores accumulation pairs
                        for hh in range(2):
                            nc.tensor.matmul(
                                s4d[:, hh, :],
                                xkt_t[:, :, c1 * KC : (c1 + 1) * KC],
                                qw8[h0 + hh],
                                start=True, stop=True, perf_mode=DR,
                            )
                        for hh in range(2):
                            nc.tensor.matmul(
                                s4a[:, hh, :],
                                xkt_t[:, :, c0 * KC : (c0 + 1) * KC],
                                qw8[h0 + hh],
                                start=False, stop=True, perf_mode=DR,
                            )
                        nc.vector._custom_dve(
                            EXP_OP, out=em_cur[:, 1], in0=s4d,
                            in1=mask_sb[:, c1, :],
                            s0=POLY_B[0], s1=POLY_B[1], imm2=POLY_B[2],
                        )
                        nc.scalar.activation(
                            em_cur[:, 0], s4a, AF.Exp, bias=0.0, scale=0.125
                        )
                        for hh in range(2):
                            nc.tensor.matmul(
                                avP[hh],
                                vaug[:, h0 + hh, 2 * pair : 2 * pair + 2, 0 : A + 1],
                                em_cur[:, :, hh, :],
                                start=(pair == 0), stop=(pair == NPAIR - 1),
                                perf_mode=DR,
                            )

                    for hh in range(2):
                        eng_copy((nc.scalar, nc.vector)[hh], nh[h0 + hh], avP[hh])

            # ---------------- epilogue ----------------
            with (
                tc.psum_pool(name="pe", bufs=1) as pm,
                tc.sbuf_pool(name="es", bufs=1) as ms,
            ):
                gt_ps = pm.tile([128, 4 * H], F32, tag="gt", bufs=1)
                for qtile in range(4):
                    nc.tensor.transpose(
                        gt_ps[:, qtile * H : qtile * H + H],
                        gates[:, qtile * 128 : (qtile + 1) * 128],
                        identity[:H, :H],
                    )
                gt_sb = ms.tile([128, 4 * H], F32, tag="gtsb", bufs=1)
                nc.any.tensor_copy(gt_sb, gt_ps)
                boB_ps = pm.tile([128, DO], F32, tag="bob", bufs=1)
                nc.tensor.matmul(boB_ps, ones1, bo_t, start=True, stop=True)
                boB = ms.tile([128, DO], F32, tag="bobsb", bufs=1)
                nc.any.tensor_copy(boB, boB_ps)
                for qtile in range(4):
                    acc = boB
                    for h in range(H):
                        p_ps = pm.tile([128, DO + 1], F32, tag="p", bufs=2)
                        nc.tensor.matmul(
                            p_ps,
                            nh[h][:, qtile * 128 : (qtile + 1) * 128],
                            woaug,
                            start=True, stop=True,
                        )
                        rden = ms.tile([128, 1], F32, tag="rden", bufs=2)
                        nc.vector.reciprocal(rden, p_ps[:, DO : DO + 1])
                        sc = ms.tile([128, 1], F32, tag="sc", bufs=2)
                        nc.any.tensor_mul(
                            sc, rden, gt_sb[:, qtile * H + h : qtile * H + h + 1]
                        )
                        nxt = ms.tile([128, DO], F32, tag=f"acc{h % 2}", bufs=2)
                        nc.vector.scalar_tensor_tensor(
                            nxt, p_ps[:, :DO], sc, acc,
                            op0=ALU.mult, op1=ALU.add,
                        )
                        acc = nxt
                    nc.sync.dma_start(
                        out[qtile * 128 : (qtile + 1) * 128, :], acc
                    )
    nc.finalize()
    return nc


# ---------------------------------------------------------------------------
# host-side input prep
# ---------------------------------------------------------------------------
def _to_f8(x):
    import ml_dtypes
    return np.ascontiguousarray(np.asarray(x, dtype=np.float32).astype(
        ml_dtypes.float8_e4m3fn))


def _to_bf16(x):
    import ml_dtypes
    return np.ascontiguousarray(np.asarray(x, dtype=np.float32).astype(
        ml_dtypes.bfloat16))


def _dr_c_layout(xT):
    """[C=256, N] -> [128, 2, N] with c = i*128 + p."""
    return np.ascontiguousarray(xT.reshape(2, 128, -1).transpose(1, 0, 2))


def _prep_shared(x_K, Wq, Wk, Wv, Wg, bg, Wo, bo):
    xkt = x_K.T  # [256, NK]
    xkt8 = _to_f8(_dr_c_layout(xkt))

    # wqb[p, i, h, a] = Wq[h, i*128+p, a]
    wqb = _to_bf16(Wq.transpose(1, 0, 2).reshape(2, 128, H, A).transpose(1, 0, 2, 3))
    # wkTb[a, half, h, m] = Wk[h, 128*half + m, a]
    wkTb = _to_bf16(
        Wk.reshape(H, 2, 128, A).transpose(3, 1, 0, 2)
    )
    arr = np.empty((128, 2, H * A), np.float32)
    for h in range(H):
        arr[:, :, h * A:(h + 1) * A] = Wv[h].reshape(2, 128, A).transpose(1, 0, 2)
    wv8 = _to_f8(arr)
    wgtb = _to_bf16(Wg.T.reshape(2, 128, H).transpose(1, 0, 2))
    return {
        "xkt8": xkt8, "wqb": wqb, "wkTb": wkTb, "wv8": wv8, "wgtb": wgtb,
        "bg": np.asarray(bg, np.float32).reshape(H, 1),
        "wo": np.ascontiguousarray(np.asarray(Wo, np.float32)),
        "bo": np.asarray(bo, np.float32).reshape(1, DO),
    }


def _prep_mask_core(mask_sl):
    """mask_sl: [NQC, NK] int32 -> maskx [NKC, 128, 2*NQC] fp8 per-chunk layout."""
    import ml_dtypes
    mt = mask_sl.T.astype(np.float32)  # [NK, NQC]
    maskx = np.zeros((NKC, 128, 2 * NQC), np.float32)
    for c in range(NKC):
        blk = mt[c * KC:(c + 1) * KC]  # [128, NQC]
        if CHUNK_IS_ACT[c]:
            # rows p<64: col i*512+q holds bias of key k = i*64+p
            bias = (blk - 1.0) * (-MASK_BIAS)
            maskx[c, 0:64, 0:NQC] = bias[0:64]
            maskx[c, 0:64, NQC:] = bias[64:128]
        else:
            maskx[c, :, 0:NQC] = blk
            maskx[c, :, NQC:] = blk
    return np.ascontiguousarray(maskx.astype(ml_dtypes.float8_e4m3fn))


def kernel(x_Q, x_K, mask, Wq, Wk, Wv, Wg, bg, Wo, bo):
    from concourse.bass_utils import run_bass_kernel_spmd

    x_Q = np.asarray(x_Q, dtype=np.float32)
    x_K = np.asarray(x_K, dtype=np.float32)
    mask = np.asarray(mask, dtype=np.int32)

    shared = _prep_shared(
        x_K, np.asarray(Wq, np.float32), np.asarray(Wk, np.float32),
        np.asarray(Wv, np.float32), np.asarray(Wg, np.float32),
        bg, Wo, bo,
    )

    in_maps = []
    for cidx in range(NCORES):
        sl = slice(cidx * NQC, (cidx + 1) * NQC)
        xqt = x_Q[sl].T  # [256, NQC]
        m = {
            "xqtb": _to_bf16(_dr_c_layout(xqt)),
            "maskx": _prep_mask_core(mask[sl]),
        }
        m.update(shared)
        in_maps.append(m)

    if "nc" not in _cache:
        _cache["nc"] = _build_kernel()
    res = run_bass_kernel_spmd(
        _cache["nc"], in_maps, list(range(NCORES)),
        trace=bool(int(os.environ.get("BASS_KERNEL_TRACE", "0"))),
    )
    if res.exec_time_ns is not None:
        print(f"HW exec time: {res.exec_time_ns} ns")
    return np.concatenate([r["out"] for r in res.results], axis=0)
